# revision 8
# baseline (speedup 1.0000x reference)
"""CluStream (vq_codebook) Trainium2 kernel.

reference semantics (B=16384, K=1024, D=512, T=0.07):
    sq      = ||z||^2 + ||c||^2 - 2 z@c.T          # [B, K]
    closest = argmin_k sqrt(max(sq, 0))            # [B] int32
    z_n     = z / max(||z||, 1e-12)                # [B, D]
    logits  = (z_n @ c.T) / T
    loss    = -mean_b log_softmax(logits)[b, closest[b]]
    returns (z_n, closest, loss)

Strategy (data-parallel over 8 cores, 2048 rows each):
  host: transpose z/c once, precompute row norms, cn2/2, index-encoding table.
  device (per core, all engines balanced near the ~30us roofline):
    PE    : s = z @ c.T as fp32r matmuls (lhsT = zT chunk, rhs = cT chunk),
            plus bit-exact fp32 transposes of z_n back to natural layout.
    ACT   : rounds zT chunks to the fp32r grid; exp(s * 1/(norm*T)) with
            per-row scale, accumulating sum(exp) per row.
    DVE   : fused (s - cn2/2) -> SBUF with max-reduce accum -> v1 per row;
            z_n^T = zT * (1/norm) broadcast multiply.
    GPSIMD: candidate scan: sum over k of (1024*k+1) * [pscore_k >= v1-tau].
  host: decode argmin (+ exact fp64 rescue of rows with >1 candidate within
  tau — covers the fp32r rounding noise), assemble loss from per-row
  sum(exp), v1 and cn2, and stitch outputs.
"""
import numpy as np

import concourse.bacc as bacc
import concourse.tile as tile
import concourse.mybir as mybir
from concourse.bass_utils import run_bass_kernel_spmd

dt = mybir.dt

B_FULL, D, K = 16384, 512, 1024
N_CORES = 8
B = B_FULL // N_CORES          # 2048 rows per core
NBT = B // 128                 # 16 b-tiles per core
TEMPERATURE = 0.07
TAU = 0.15                     # rescue threshold on pscore = s - cn2/2
OFFSET = 256.0                 # centering constant folded into negc
VARIANT = "sub"                # "sub": DVE subtract + fp16 scans; "fold": PE fold + f32 PSUM scans
EXP_BIAS = -48.0               # exp(logit + EXP_BIAS): keeps sumexp inside f32 range
ENC_STRIDE = float(K)          # candidate encoding: sum(K*k + 1)

_PROGRAM_CACHE = {}


def _build_program():
    if "nc" in _PROGRAM_CACHE:
        return _PROGRAM_CACHE["nc"]
    nc = bacc.Bacc("TRN2", target_bir_lowering=False, debug=False)

    zT_d = nc.dram_tensor("zT", [D, B], dt.float32, kind="ExternalInput").ap()
    cT_d = nc.dram_tensor("cT", [D, K], dt.float32, kind="ExternalInput").ap()
    negc_d = nc.dram_tensor("negc", [1, K], dt.float32, kind="ExternalInput").ap()
    ones_d = nc.dram_tensor("ones", [1, 128], dt.float32, kind="ExternalInput").ap()
    scale_d = nc.dram_tensor("scale_pt", [128, NBT], dt.float32,
                             kind="ExternalInput").ap()
    rnorm_d = nc.dram_tensor("rnorm", [1, B], dt.float32, kind="ExternalInput").ap()

    znT_d = nc.dram_tensor("znT", [D, B], dt.float32, kind="ExternalOutput").ap()
    v8_d = nc.dram_tensor("v8", [128, 8 * NBT],
                          dt.float32 if VARIANT == "fold" else dt.float16,
                          kind="ExternalOutput").ap()
    i8_d = nc.dram_tensor("i8", [128, 8 * NBT], dt.uint32,
                          kind="ExternalOutput").ap()
    sexp_d = nc.dram_tensor("sumexp", [128, NBT], dt.float32,
                            kind="ExternalOutput").ap()

    with tile.TileContext(nc) as tc:
        with (
            tc.tile_pool(name="const", bufs=1) as const,
            tc.tile_pool(name="zt", bufs=1) as ztp,
            tc.tile_pool(name="work", bufs=2) as work,
            tc.tile_pool(name="psum", bufs=2, space="PSUM") as psum,
        ):
            # ---------------- constants / broadcasts ----------------
            scale_t = const.tile([128, NBT], dt.float32, tag="scale")
            nc.sync.dma_start(scale_t[:], scale_d[:, :])
            ebias_t = const.tile([128, 1], dt.float32, tag="ebias")
            nc.vector.memset(ebias_t[:], EXP_BIAS)

            if VARIANT == "fold":
                negc_row = const.tile([1, K], dt.float32, tag="negc_row")
                nc.sync.dma_start(negc_row[:], negc_d[:, :])
                negc_r = const.tile([1, K], dt.float32r, tag="negc_r")
                nc.gpsimd.tensor_copy(negc_r[:], negc_row[:])
                ones_row = const.tile([1, 128], dt.float32, tag="ones_row")
                nc.sync.dma_start(ones_row[:], ones_d[:, :])
                ones_r = const.tile([1, 128], dt.float32r, tag="ones_r")
                nc.gpsimd.tensor_copy(ones_r[:], ones_row[:])
            else:
                negc_row = const.tile([1, K], dt.float32, tag="negc_row")
                nc.sync.dma_start(negc_row[:], negc_d[:, :])
                negc_bc = const.tile([128, K], dt.float32, tag="negc_bc")
                nc.gpsimd.partition_broadcast(negc_bc[:], negc_row[:])

            rnorm_row = const.tile([1, B], dt.float32, tag="rnorm_row")
            nc.sync.dma_start(rnorm_row[:], rnorm_d[:, :])
            rnorm_bc = const.tile([128, B], dt.float32, tag="rnorm_bc")
            nc.gpsimd.partition_broadcast(rnorm_bc[:], rnorm_row[:])

            # ---------------- cT load + fp32r rounding ----------------
            ctr_tiles = []
            for d in range(4):
                ct = const.tile([128, K], dt.float32, tag=f"ct{d}")
                nc.sync.dma_start(ct[:], cT_d[d * 128:(d + 1) * 128, :])
                ctr = const.tile([128, K], dt.float32r, tag=f"ctr{d}")
                for kb in range(2):
                    sl = slice(kb * 512, (kb + 1) * 512)
                    nc.gpsimd.tensor_copy(ctr[:, sl], ct[:, sl])
                ctr_tiles.append(ctr)

            # ---------------- zT load + fp32r rounding + z_n^T ----------------
            zt_tiles, ztr_tiles, znT_tiles = [], [], []
            for d in range(4):
                zt = ztp.tile([128, B], dt.float32, tag=f"zt{d}")
                ztr = ztp.tile([128, B], dt.float32r, tag=f"ztr{d}")
                znT = ztp.tile([128, B], dt.float32, tag=f"znT{d}")
                for bb in range(4):
                    sl = slice(bb * 512, (bb + 1) * 512)
                    nc.sync.dma_start(zt[:, sl], zT_d[d * 128:(d + 1) * 128, sl])
                    nc.scalar.copy(ztr[:, sl], zt[:, sl])
                for bb in range(2):
                    sl = slice(bb * 1024, (bb + 1) * 1024)
                    nc.gpsimd.tensor_mul(znT[:, sl], zt[:, sl], rnorm_bc[:, sl])
                    nc.sync.dma_start(znT_d[d * 128:(d + 1) * 128, sl], znT[:, sl])
                zt_tiles.append(zt)
                ztr_tiles.append(ztr)
                znT_tiles.append(znT)

            # per-row outputs
            v8_dt = dt.float32 if VARIANT == "fold" else dt.float16
            v8_t = const.tile([128, 8 * NBT], v8_dt, tag="v8")
            i8_t = const.tile([128, 8 * NBT], dt.uint32, tag="i8")
            sexp_t = const.tile([128, NBT], dt.float32, tag="sexp")

            # ---------------- main loop over 16 b-tiles ----------------
            for t in range(NBT):
                bsl = slice(t * 128, (t + 1) * 128)
                s_ps = psum.tile([128, K], dt.float32, tag="s")
                for kb in range(2):
                    ksl = slice(kb * 512, (kb + 1) * 512)
                    for d in range(4):
                        nc.tensor.matmul(
                            s_ps[:, ksl], ztr_tiles[d][:, bsl],
                            ctr_tiles[d][:, ksl],
                            start=(d == 0),
                            stop=(VARIANT == "sub" and d == 3),
                        )

                # ACT: exp(s * scale_row) with row-sum accumulation
                esc = work.tile([128, K], dt.float32, tag="esc")
                nc.scalar.activation(
                    esc[:], s_ps[:], mybir.ActivationFunctionType.Exp,
                    bias=ebias_t[:, :], scale=scale_t[:, t:t + 1],
                    accum_out=sexp_t[:, t:t + 1],
                )

                if VARIANT == "fold":
                    # PE: fold s -> pscore = s + (256 - cn2/2) in place (after
                    # the exp read; Tile's WAR dep serializes the fold behind it)
                    for kb in range(2):
                        ksl = slice(kb * 512, (kb + 1) * 512)
                        nc.tensor.matmul(
                            s_ps[:, ksl], ones_r[:, :], negc_r[:, ksl],
                            start=False, stop=True, skip_group_check=True,
                        )
                    nc.vector.max(v8_t[:, t * 8:(t + 1) * 8], s_ps[:])
                    nc.vector.max_index(i8_t[:, t * 8:(t + 1) * 8],
                                        v8_t[:, t * 8:(t + 1) * 8], s_ps[:])
                else:
                    # DVE: pscore = s + (256 - cn2/2) -> fp16 SBUF, then fp16 scans
                    pscore = work.tile([128, K], dt.float16, tag="pscore")
                    nc.vector.tensor_add(pscore[:], s_ps[:], negc_bc[:])
                    nc.vector.max(v8_t[:, t * 8:(t + 1) * 8], pscore[:])
                    nc.vector.max_index(i8_t[:, t * 8:(t + 1) * 8],
                                        v8_t[:, t * 8:(t + 1) * 8], pscore[:])


            nc.sync.dma_start(v8_d[:, :], v8_t[:])
            nc.sync.dma_start(i8_d[:, :], i8_t[:])
            nc.sync.dma_start(sexp_d[:, :], sexp_t[:])

    nc.compile()
    _PROGRAM_CACHE["nc"] = nc
    return nc


def _host_prep(z, clusters):
    z = np.ascontiguousarray(z, dtype=np.float32)
    c = np.ascontiguousarray(clusters, dtype=np.float32)
    zT = np.ascontiguousarray(z.T)                     # [D, B_FULL]
    cT = np.ascontiguousarray(c.T)                     # [D, K]
    cn2 = (c.astype(np.float64) ** 2).sum(1)
    negc = (OFFSET - cn2 / 2.0).astype(np.float32)
    ones128 = np.ones((1, 128), dtype=np.float32)
    zz = (z.astype(np.float64) ** 2).sum(1)
    norm = np.maximum(np.sqrt(zz), 1e-12)
    rnorm = (1.0 / norm).astype(np.float32)            # [B_FULL]
    scale_row = (1.0 / (norm * TEMPERATURE)).astype(np.float32)

    in_maps = []
    for i in range(N_CORES):
        sl = slice(i * B, (i + 1) * B)
        in_maps.append({
            "zT": np.ascontiguousarray(zT[:, sl]),
            "cT": cT,
            "negc": negc[None, :],
            "ones": ones128,
            "scale_pt": np.ascontiguousarray(scale_row[sl].reshape(NBT, 128).T),
            "rnorm": rnorm[None, sl],
        })
    aux = dict(z=z, c=c, cn2=cn2, norm=norm, scale_row=scale_row)
    return in_maps, aux


def _col_major_rows(a):
    """[128, NBT] per-tile-column layout -> [B] row order."""
    return np.ascontiguousarray(a.T).reshape(-1)


def _host_finish(results, aux):
    z, c, cn2 = aux["z"], aux["c"], aux["cn2"]
    v8 = np.concatenate([r["v8"].T.reshape(NBT, 8, 128).transpose(0, 2, 1)
                         .reshape(B, 8) for r in results])
    i8 = np.concatenate([r["i8"].T.reshape(NBT, 8, 128).transpose(0, 2, 1)
                         .reshape(B, 8) for r in results]).astype(np.int64)
    sumexp = np.concatenate([_col_major_rows(r["sumexp"]) for r in results])
    zn = np.concatenate([np.ascontiguousarray(r["znT"].T) for r in results],
                        axis=0)

    closest = np.clip(i8[:, 0], 0, K - 1)
    v1 = v8[:, 0].astype(np.float32)
    s_at = (v1 - OFFSET) + cn2.astype(np.float32)[closest] * 0.5

    scale_row = aux["scale_row"].astype(np.float64)
    with np.errstate(divide="ignore", invalid="ignore"):
        lse = np.log(sumexp.astype(np.float64)) - EXP_BIAS
    logit_at = s_at.astype(np.float64) * scale_row

    bad = np.nonzero((v8[:, 0].astype(np.float32) - v8[:, 1] < TAU)
                     | (i8[:, 0] >= K) | ~np.isfinite(lse))[0]
    if bad.size:
        zb = z[bad].astype(np.float64)
        s_b = zb @ c.astype(np.float64).T               # [nb, K]
        sq = (zb ** 2).sum(1)[:, None] + cn2[None, :] - 2.0 * s_b
        pick = np.argmin(sq, axis=1)
        closest[bad] = pick
        lg = s_b * scale_row[bad, None]
        m = lg.max(1, keepdims=True)
        lse[bad] = m[:, 0] + np.log(np.exp(lg - m).sum(1))
        logit_at[bad] = lg[np.arange(bad.size), pick]

    loss = np.float32(np.mean(lse - logit_at))
    return zn, closest.astype(np.int32), loss


def _run(z, clusters, trace=False, trace_kwargs=None):
    nc = _build_program()
    in_maps, aux = _host_prep(z, clusters)
    kw = {}
    if trace:
        kw.update(trace=True, trace_kwargs=trace_kwargs or {})
    res = run_bass_kernel_spmd(nc, in_maps, list(range(N_CORES)), **kw)
    return _host_finish(res.results, aux), res


def kernel(z, clusters):
    out, _ = _run(z, clusters)
    return out


if __name__ == "__main__":
    rng = np.random.default_rng(0)
    z = rng.standard_normal((B_FULL, D), dtype=np.float32)
    c = rng.standard_normal((K, D), dtype=np.float32)
    out = kernel(z, c)
    print([getattr(o, "shape", o) for o in out])


# revision 10
# speedup vs baseline: 1.3187x; 1.3187x over previous
"""CluStream (vq_codebook) Trainium2 kernel.

reference semantics (B=16384, K=1024, D=512, T=0.07):
    sq      = ||z||^2 + ||c||^2 - 2 z@c.T          # [B, K]
    closest = argmin_k sqrt(max(sq, 0))            # [B] int32
    z_n     = z / max(||z||, 1e-12)                # [B, D]
    logits  = (z_n @ c.T) / T
    loss    = -mean_b log_softmax(logits)[b, closest[b]]
    returns (z_n, closest, loss)

Data-parallel over 8 cores (2048 rows each). Host pre-transposes z/c and
precomputes row norms; device computes s = z@c.T in fp32r on the PE,
exp+row-sum on ACT, pscore = s + (256 - cn2/2) in fp16 plus row-max and a
fused candidate scan (sum of (1+k/1024)*[pscore >= v1-tau]) on DVE, and
z_n^T on GPSIMD. Host decodes the argmin (code in [1,2) <=> unique
candidate), rescues ambiguous/overflowed rows exactly in fp64, and
assembles the loss from per-row sum(exp), v1 and cn2.
"""
import numpy as np

import concourse.bacc as bacc
import concourse.tile as tile
import concourse.mybir as mybir
from concourse.bass_utils import run_bass_kernel_spmd

dt = mybir.dt

B_FULL, D, K = 16384, 512, 1024
N_CORES = 8
B = B_FULL // N_CORES          # 2048 rows per core
NBT = B // 128                 # 16 b-tiles per core
TEMPERATURE = 0.07
TAU = 0.15                     # rescue threshold on pscore = s - cn2/2 (+const)
OFFSET = 256.0                 # centering constant folded into negc
EXP_BIAS = -48.0               # exp(logit + EXP_BIAS): keep sumexp in f32 range

_PROGRAM_CACHE = {}


def _build_program():
    if "nc" in _PROGRAM_CACHE:
        return _PROGRAM_CACHE["nc"]
    nc = bacc.Bacc("TRN2", target_bir_lowering=False, debug=False)

    zT_d = nc.dram_tensor("zT", [D, B], dt.float32, kind="ExternalInput").ap()
    cT_d = nc.dram_tensor("cT", [D, K], dt.float32, kind="ExternalInput").ap()
    negc_d = nc.dram_tensor("negc", [1, K], dt.float32, kind="ExternalInput").ap()
    enc_d = nc.dram_tensor("enc", [1, K], dt.float16, kind="ExternalInput").ap()
    scale_d = nc.dram_tensor("scale_pt", [128, NBT], dt.float32,
                             kind="ExternalInput").ap()
    rnorm_d = nc.dram_tensor("rnorm", [1, B], dt.float32, kind="ExternalInput").ap()

    znT_d = nc.dram_tensor("znT", [D, B], dt.float32, kind="ExternalOutput").ap()
    v1_d = nc.dram_tensor("v1", [128, NBT], dt.float32, kind="ExternalOutput").ap()
    code_d = nc.dram_tensor("code", [128, NBT], dt.float32,
                            kind="ExternalOutput").ap()
    sexp_d = nc.dram_tensor("sumexp", [128, NBT], dt.float32,
                            kind="ExternalOutput").ap()

    with tile.TileContext(nc) as tc:
        with (
            tc.tile_pool(name="const", bufs=1) as const,
            tc.tile_pool(name="zt", bufs=1) as ztp,
            tc.tile_pool(name="work", bufs=3) as work,
            tc.tile_pool(name="psum", bufs=3, space="PSUM") as psum,
        ):
            # ------------- constants / broadcasts (startup-critical) -------------
            scale_t = const.tile([128, NBT], dt.float32, tag="scale")
            nc.sync.dma_start(scale_t[:], scale_d[:, :])
            ebias_t = const.tile([128, 1], dt.float32, tag="ebias")
            nc.vector.memset(ebias_t[:], EXP_BIAS)

            negc_row = const.tile([1, K], dt.float32, tag="negc_row")
            nc.sync.dma_start(negc_row[:], negc_d[:, :])
            negc_bc = const.tile([128, K], dt.float32, tag="negc_bc")
            nc.gpsimd.partition_broadcast(negc_bc[:], negc_row[:])
            enc_row = const.tile([1, K], dt.float16, tag="enc_row")
            nc.sync.dma_start(enc_row[:], enc_d[:, :])
            enc_bc = const.tile([128, K], dt.float16, tag="enc_bc")
            nc.gpsimd.partition_broadcast(enc_bc[:], enc_row[:])
            rnorm_row = const.tile([1, B], dt.float32, tag="rnorm_row")
            nc.sync.dma_start(rnorm_row[:], rnorm_d[:, :])
            rnorm_bc = const.tile([128, B], dt.float32, tag="rnorm_bc")
            nc.gpsimd.partition_broadcast(rnorm_bc[:], rnorm_row[:])

            # ------------- cT load + fp32r rounding (DVE 2x copies) -------------
            ctr_tiles = []
            for d in range(4):
                ct = const.tile([128, K], dt.float32, tag=f"ct{d}")
                nc.sync.dma_start(ct[:], cT_d[d * 128:(d + 1) * 128, :])
                ctr = const.tile([128, K], dt.float32r, tag=f"ctr{d}")
                for kb in range(2):
                    sl = slice(kb * 512, (kb + 1) * 512)
                    nc.vector.tensor_copy(ctr[:, sl], ct[:, sl])
                ctr_tiles.append(ctr)

            # persistent z tiles
            zt_tiles = [ztp.tile([128, B], dt.float32, tag=f"zt{d}",
                                 name=f"zt{d}") for d in range(4)]
            ztr_tiles = [ztp.tile([128, B], dt.float32r, tag=f"ztr{d}",
                                  name=f"ztr{d}") for d in range(4)]
            znT_tiles = [ztp.tile([128, B], dt.float32, tag=f"znT{d}",
                                  name=f"znT{d}") for d in range(4)]

            # per-row outputs
            v1_t = const.tile([128, NBT], dt.float32, tag="v1")
            v1mt_t = const.tile([128, NBT], dt.float32, tag="v1mt")
            code_t = const.tile([128, NBT], dt.float32, tag="code")
            sexp_t = const.tile([128, NBT], dt.float32, tag="sexp")

            # ------------- pipelined over 4 b-blocks of 512 rows -------------
            for bb in range(4):
                sl512 = slice(bb * 512, (bb + 1) * 512)
                for d in range(4):
                    nc.sync.dma_start(zt_tiles[d][:, sl512],
                                      zT_d[d * 128:(d + 1) * 128, sl512])
                    nc.scalar.copy(ztr_tiles[d][:, sl512], zt_tiles[d][:, sl512])
                # z_n^T on GPSIMD, stores on the ACT DGE queue
                for d in range(4):
                    nc.gpsimd.tensor_mul(znT_tiles[d][:, sl512],
                                         zt_tiles[d][:, sl512],
                                         rnorm_bc[:, sl512])
                    nc.scalar.dma_start(znT_d[d * 128:(d + 1) * 128, sl512],
                                        znT_tiles[d][:, sl512])

                for t in range(bb * 4, bb * 4 + 4):
                    bsl = slice(t * 128, (t + 1) * 128)
                    s_ps = psum.tile([128, K], dt.float32, tag="s")
                    for kb in range(2):
                        ksl = slice(kb * 512, (kb + 1) * 512)
                        for d in range(4):
                            nc.tensor.matmul(
                                s_ps[:, ksl], ztr_tiles[d][:, bsl],
                                ctr_tiles[d][:, ksl],
                                start=(d == 0), stop=(d == 3),
                            )

                    # ACT: exp(s*scale + bias) with row-sum accumulation
                    esc = work.tile([128, K], dt.float32, tag="esc")
                    nc.scalar.activation(
                        esc[:], s_ps[:], mybir.ActivationFunctionType.Exp,
                        bias=ebias_t[:, :], scale=scale_t[:, t:t + 1],
                        accum_out=sexp_t[:, t:t + 1],
                    )

                    # DVE: pscore (fp16), row max, candidate-code scan
                    pscore = work.tile([128, K], dt.float16, tag="pscore")
                    nc.vector.tensor_add(pscore[:], s_ps[:], negc_bc[:])
                    nc.vector.reduce_max(v1_t[:, t:t + 1], pscore[:],
                                         axis=mybir.AxisListType.X)
                    nc.vector.tensor_scalar_add(v1mt_t[:, t:t + 1],
                                                v1_t[:, t:t + 1], -TAU)
                    gsc = work.tile([128, K], dt.float16, tag="gsc")
                    nc.vector.scalar_tensor_tensor(
                        out=gsc[:], in0=pscore[:], scalar=v1mt_t[:, t:t + 1],
                        in1=enc_bc[:],
                        op0=mybir.AluOpType.is_ge, op1=mybir.AluOpType.mult,
                        accum_out=code_t[:, t:t + 1],
                    )

            nc.scalar.dma_start(v1_d[:, :], v1_t[:])
            nc.scalar.dma_start(code_d[:, :], code_t[:])
            nc.scalar.dma_start(sexp_d[:, :], sexp_t[:])

    nc.compile()
    _PROGRAM_CACHE["nc"] = nc
    return nc


def _host_prep(z, clusters):
    z = np.ascontiguousarray(z, dtype=np.float32)
    c = np.ascontiguousarray(clusters, dtype=np.float32)
    zT = np.ascontiguousarray(z.T)                     # [D, B_FULL]
    cT = np.ascontiguousarray(c.T)                     # [D, K]
    cn2 = (c.astype(np.float64) ** 2).sum(1)
    negc = (OFFSET - cn2 / 2.0).astype(np.float32)
    enc = (1.0 + np.arange(K) / 1024.0).astype(np.float16)
    zz = (z.astype(np.float64) ** 2).sum(1)
    norm = np.maximum(np.sqrt(zz), 1e-12)
    rnorm = (1.0 / norm).astype(np.float32)            # [B_FULL]
    scale_row = (1.0 / (norm * TEMPERATURE)).astype(np.float32)

    in_maps = []
    for i in range(N_CORES):
        sl = slice(i * B, (i + 1) * B)
        in_maps.append({
            "zT": np.ascontiguousarray(zT[:, sl]),
            "cT": cT,
            "negc": negc[None, :],
            "enc": enc[None, :],
            "scale_pt": np.ascontiguousarray(scale_row[sl].reshape(NBT, 128).T),
            "rnorm": rnorm[None, sl],
        })
    aux = dict(z=z, c=c, cn2=cn2, norm=norm, scale_row=scale_row)
    return in_maps, aux


def _col_major_rows(a):
    """[128, NBT] per-tile-column layout -> [B] row order."""
    return np.ascontiguousarray(a.T).reshape(-1)


def _host_finish(results, aux):
    z, c, cn2 = aux["z"], aux["c"], aux["cn2"]
    v1 = np.concatenate([_col_major_rows(r["v1"]) for r in results])
    code = np.concatenate([_col_major_rows(r["code"]) for r in results])
    sumexp = np.concatenate([_col_major_rows(r["sumexp"]) for r in results])
    zn = np.concatenate([np.ascontiguousarray(r["znT"].T) for r in results],
                        axis=0)

    codef = code.astype(np.float64)
    idx = np.round((codef - 1.0) * 1024.0).astype(np.int64)
    closest = np.clip(idx, 0, K - 1)
    s_at = (v1 - OFFSET) + cn2.astype(np.float32)[closest] * 0.5

    scale_row = aux["scale_row"].astype(np.float64)
    with np.errstate(divide="ignore", invalid="ignore"):
        lse = np.log(sumexp.astype(np.float64)) - EXP_BIAS
    logit_at = s_at.astype(np.float64) * scale_row

    bad = np.nonzero((codef < 0.999) | (codef > 1.9995) | (idx < 0)
                     | (idx >= K) | ~np.isfinite(lse))[0]
    if bad.size:
        zb = z[bad].astype(np.float64)
        s_b = zb @ c.astype(np.float64).T               # [nb, K]
        sq = (zb ** 2).sum(1)[:, None] + cn2[None, :] - 2.0 * s_b
        pick = np.argmin(sq, axis=1)
        closest[bad] = pick
        lg = s_b * scale_row[bad, None]
        m = lg.max(1, keepdims=True)
        lse[bad] = m[:, 0] + np.log(np.exp(lg - m).sum(1))
        logit_at[bad] = lg[np.arange(bad.size), pick]

    loss = np.float32(np.mean(lse - logit_at))
    return zn, closest.astype(np.int32), loss


def _run(z, clusters, trace=False, trace_kwargs=None):
    nc = _build_program()
    in_maps, aux = _host_prep(z, clusters)
    kw = {}
    if trace:
        kw.update(trace=True, trace_kwargs=trace_kwargs or {})
    res = run_bass_kernel_spmd(nc, in_maps, list(range(N_CORES)), **kw)
    return _host_finish(res.results, aux), res


def kernel(z, clusters):
    out, _ = _run(z, clusters)
    return out


if __name__ == "__main__":
    rng = np.random.default_rng(0)
    z = rng.standard_normal((B_FULL, D), dtype=np.float32)
    c = rng.standard_normal((K, D), dtype=np.float32)
    out = kernel(z, c)
    print([getattr(o, "shape", o) for o in out])
